# revision 27
# baseline (speedup 1.0000x reference)
"""Trainium2 Bass kernel for nn_BlockModel_82678120448388.

Model: per (batch, head): 8x8 transition matrices from an MLP (normalized),
values from a second MLP, then a linear recurrence s_t = A_t s_{t-1} + v_t
over seq=2048.

Sharding: 8 cores = 4 batches x 2 head-halves (32 heads each). Weights
replicated / row-sliced on host; full inputs in, full output out.

Scan strategy: the normalized A_t are strongly contractive (product over a
16-token window has norm ~1e-5), so the recurrence is chunk-local to far
below the error tolerance. Each core runs K=16 independent chunk scans of
C=128 tokens in partition-parallel, each warmed up with the last W=16
tokens of the previous chunk from a zero state; chunk 0 starts exactly
from a0.

DMA layout: A and v for token (chunk c, pos p) are stored contiguously in
av_dram[(c,ho), p, 288] so one DMA gathers 8 scan steps; W2 is host-
re-laid-out so each (q, n) slab loads in 4 large DMAs; x is host-striped
so each q loads in one DMA.
"""

import numpy as np
import ml_dtypes
from contextlib import ExitStack

import concourse.bass as bass
import concourse.bacc as bacc
import concourse.tile as tile
from concourse import mybir

F32 = mybir.dt.float32
BF16 = mybir.dt.bfloat16
AF = mybir.ActivationFunctionType
ALU = mybir.AluOpType

BS, SEQ, EMB, BD = 4, 2048, 512, 8
H = EMB // BD      # 64 global heads
HL = 32            # heads per core
NF = HL * BD * BD  # 2048 blk feats per core
VF = HL * BD       # 256 v feats per core
HID = EMB * BD     # 4096
P = 128

N_CORES = 8

K = 16             # chunks per core
C = SEQ // K       # 128 tokens per chunk
W = 16             # warm-up tokens per chunk
NHO = P // K       # 8 head-groups on partitions
NHR = HL // NHO    # 4 heads per group in free dim
HRI = NHR * BD     # 32
AVW = NHR * BD * 9  # 288: per (c,ho,pos): (hr, i, [A row | v]) 9-wide rows
ROWW = C * AVW     # av_dram row size per (c, ho)


def _rot(tau):
    """Within-chunk position of the first token in MLP tile tau.

    Warm-up positions [112, 128) are produced by tiles 0-1 so the scan's
    warm-up steps only depend on the first q's MLP output.
    """
    return (112 + 8 * tau) % C


def _tau_of(pos):
    return (pos - 112) // 8 if pos >= 112 else pos // 8 + 2


def build_nc(TOK=SEQ, scan_steps=None):
    QT = 512
    NQ = TOK // QT
    TPQ = QT // P

    nc = bacc.Bacc("TRN2", target_bir_lowering=False, debug=False)

    # xs[k, p, q, col]: pre-striped x so each q loads in one DMA
    xs = nc.dram_tensor("xs", [4 * P * NQ * QT], BF16, kind="ExternalInput")
    w1 = nc.dram_tensor("w1", [EMB, HID], BF16, kind="ExternalInput")
    b1 = nc.dram_tensor("b1", [HID, 1], F32, kind="ExternalInput")
    # w2n[n, hid, f]: per-n contiguous slabs
    w2 = nc.dram_tensor("w2", [(NF // 512) * HID * 512], BF16, kind="ExternalInput")
    b2 = nc.dram_tensor("b2", [P, NF], BF16, kind="ExternalInput")
    v1 = nc.dram_tensor("v1", [EMB, EMB], BF16, kind="ExternalInput")
    c1 = nc.dram_tensor("c1", [EMB, 1], F32, kind="ExternalInput")
    v2 = nc.dram_tensor("v2", [EMB, VF], BF16, kind="ExternalInput")
    c2 = nc.dram_tensor("c2", [1, VF], BF16, kind="ExternalInput")
    a0 = nc.dram_tensor("a0", [NHO, HRI], BF16, kind="ExternalInput")
    out = nc.dram_tensor("out", [P, C * HRI], BF16, kind="ExternalOutput")

    av_dram = nc.dram_tensor("av_scratch", [P * ROWW], BF16)

    with ExitStack() as ctx:
        tc = ctx.enter_context(tile.TileContext(nc))
        cpool = ctx.enter_context(tc.tile_pool(name="consts", bufs=1))
        wpool = ctx.enter_context(tc.tile_pool(name="weights", bufs=1))
        xpool = ctx.enter_context(tc.tile_pool(name="xstream", bufs=2))
        hpool = ctx.enter_context(tc.tile_pool(name="hidden", bufs=1))
        hvpool = ctx.enter_context(tc.tile_pool(name="hv", bufs=2))
        w2pool = ctx.enter_context(tc.tile_pool(name="w2stream", bufs=2))
        l1ps = ctx.enter_context(tc.tile_pool(name="l1ps", bufs=2, space="PSUM"))
        l2ps = ctx.enter_context(tc.tile_pool(name="l2ps", bufs=TPQ, space="PSUM"))
        vps = ctx.enter_context(tc.tile_pool(name="vps", bufs=1, space="PSUM"))
        blkpool = ctx.enter_context(tc.tile_pool(name="blk", bufs=2 * TPQ))
        pwpool = ctx.enter_context(tc.tile_pool(name="pw", bufs=4))
        atpool = ctx.enter_context(tc.tile_pool(name="at", bufs=4))
        smpool = ctx.enter_context(tc.tile_pool(name="small", bufs=4))
        agpool = ctx.enter_context(tc.tile_pool(name="agather", bufs=2))
        mopool = ctx.enter_context(tc.tile_pool(name="multout", bufs=2))
        srpool = ctx.enter_context(tc.tile_pool(name="sred", bufs=2))
        scpool = ctx.enter_context(tc.tile_pool(name="scan", bufs=1))

        # ---- constants / weights ----
        ones_s = cpool.tile([1, P], BF16, tag="ones")
        nc.vector.memset(ones_s[:], 1.0)
        b1_s = cpool.tile([P, HID // P], F32, tag="b1")
        nc.sync.dma_start(b1_s[:], b1[:].rearrange("(m p) one -> p (m one)", p=P))
        c1_s = cpool.tile([P, EMB // P], F32, tag="c1")
        nc.sync.dma_start(c1_s[:], c1[:].rearrange("(m p) one -> p (m one)", p=P))
        b2_s = cpool.tile([P, NF], BF16, tag="b2")
        nc.sync.dma_start(b2_s[:], b2[:, :])
        c2_s = cpool.tile([1, VF], BF16, tag="c2")
        nc.sync.dma_start(c2_s[:], c2[:])
        a0_s = cpool.tile([NHO, HRI], BF16, tag="a0")
        nc.sync.dma_start(a0_s[:], a0[:])

        v1_s = wpool.tile([P, 4, EMB], BF16, tag="v1")
        nc.sync.dma_start(v1_s[:], v1[:].rearrange("(k p) m -> p k m", p=P))
        v2_s = wpool.tile([P, 4, VF], BF16, tag="v2")
        nc.sync.dma_start(v2_s[:], v2[:].rearrange("(k p) n -> p k n", p=P))

        # ================= scan state =================
        # s_all slot w (36 per hr-group... 4*9=36 wide): cols hr*9+i = state,
        # col hr*9+8 = constant 1.0 so the fused step's 9-wide reduce adds v.
        NS = W + C
        SW = NS * NHR * 9
        s_all = scpool.tile([P, SW], BF16, tag="sall")
        nc.vector.memset(
            bass.AP(s_all.tensor, s_all[:].offset + 8, [[SW, P], [9, NS * NHR]]),
            1.0)
        s0 = scpool.tile([P, NHR * 9], BF16, tag="s0")
        nc.vector.memset(s0[:], 0.0)
        nc.vector.memset(
            bass.AP(s0.tensor, s0[:].offset + 8, [[NHR * 9, P], [9, NHR]]), 1.0)

        agv_box = {}

        def gather_group(p_first):
            """One DMA fetching 8 steps' [A|v] into [P, 8*AVW]."""
            agv = agpool.tile([P, 8 * AVW], BF16, tag="agv", name=f"agv{p_first}")
            if p_first < 0:
                pos = C + p_first
                # dummy chunk-0 partitions (state discarded via a0 below)
                nc.sync.dma_start(
                    agv[0:NHO, :],
                    bass.AP(av_dram, pos * AVW, [[ROWW, NHO], [1, 8 * AVW]]))
                nc.sync.dma_start(
                    agv[NHO:P, :],
                    bass.AP(av_dram, pos * AVW,
                            [[NHO * ROWW, K - 1], [ROWW, NHO], [1, 8 * AVW]]))
            else:
                nc.sync.dma_start(
                    agv[:], bass.AP(av_dram, p_first * AVW,
                                    [[ROWW, P], [1, 8 * AVW]]))
            agv_box[p_first] = agv

        def scan_step(p):
            w = W + p
            p_first = p - (p + W) % 8
            if (p + W) % 8 == 0:
                gather_group(p_first)
            agv = agv_box[p_first]
            off = ((p + W) % 8) * AVW

            if p == -W:
                sprev_t, sprev_off = s0, 0
            else:
                sprev_t, sprev_off = s_all, (w - 1) * NHR * 9
            # mo[(c,ho), (hr, i, 9)] = [A|v][i, :] * [s_prev[hr, :] | 1]
            mo = mopool.tile([P, AVW], BF16, tag="mo", name=f"mo{p}")
            nc.vector.tensor_tensor(
                bass.AP(mo.tensor, mo[:].offset, [[AVW, P], [1, AVW]]),
                bass.AP(agv.tensor, agv[:].offset + off, [[8 * AVW, P], [1, AVW]]),
                bass.AP(sprev_t.tensor, sprev_t[:].offset + sprev_off,
                        [[sprev_t.shape[1], P], [9, NHR], [0, BD], [1, 9]]),
                ALU.mult)
            with nc.allow_low_precision(reason="scan state in bf16"):
                nc.vector.tensor_reduce(
                    bass.AP(s_all.tensor, s_all[:].offset + w * NHR * 9,
                            [[SW, P], [9, NHR], [1, BD]]),
                    bass.AP(mo.tensor, mo[:].offset,
                            [[AVW, P], [72, NHR], [9, BD], [1, 9]]),
                    axis=mybir.AxisListType.X, op=ALU.add)
            if p == -1:
                # chunk 0 starts exactly from a0 (no warm-up): overwrite its
                # slot W-1 state after the last warm-up step wrote it.
                nc.vector.tensor_copy(
                    bass.AP(s_all.tensor, s_all[0:NHO, :].offset + (W - 1) * NHR * 9,
                            [[SW, NHO], [9, NHR], [1, BD]]),
                    bass.AP(a0_s.tensor, a0_s[:].offset,
                            [[HRI, NHO], [BD, NHR], [1, BD]]))

        def emit_out(g):
            # positions [32g, 32g+32): one DMA per hr (strided 9-wide slots)
            for hr in range(NHR):
                nc.sync.dma_start(
                    bass.AP(out, g * 32 * HRI + hr * BD,
                            [[C * HRI, P], [HRI, 32], [1, BD]]),
                    bass.AP(s_all.tensor,
                            s_all[:].offset + (W + 32 * g) * NHR * 9 + hr * 9,
                            [[SW, P], [NHR * 9, 32], [1, BD]]))

        # steps unlocked per producing q: q0 -> p in [-W, 16); q1 -> [16, 48);
        # q2 -> [48, 80); q3 -> [80, 128) (positions >= 112 use q0's tiles).
        windows = [(-W, 16), (16, 48), (48, 80), (80, C)]

        def emit_part1(q):
            """x load + L1 + v-hidden (PE + Act relu)."""
            xq = xpool.tile([P, 4, QT], BF16, tag="xq")
            nc.sync.dma_start(
                xq[:], bass.AP(xs, q * QT,
                               [[NQ * QT, P], [P * NQ * QT, 4], [1, QT]]))
            hid_t = hpool.tile([P, HID // P, QT], BF16, tag="hid")
            for m in range(HID // P):
                if m % 8 == 0:
                    w1q = w2pool.tile([P, 4, HID // 4], BF16, tag="w1q",
                                      name=f"w1q{q}_{m // 8}")
                    nc.sync.dma_start(
                        w1q[:], bass.AP(w1, (m // 8) * (HID // 4),
                                        [[HID, P], [P * HID, 4], [1, HID // 4]]))
                ps = l1ps.tile([P, QT], F32, tag="l1")
                for k in range(4):
                    nc.tensor.matmul(ps[:], w1q[:, k, bass.ts(m % 8, P)], xq[:, k, :],
                                     start=(k == 0), stop=(k == 3))
                nc.scalar.activation(hid_t[:, m, :], ps[:], AF.Relu,
                                     bias=b1_s[:, m:m + 1])
            hv_t = hvpool.tile([P, 4, QT], BF16, tag="hv", name=f"hv{q}")
            for m in range(4):
                ps = l1ps.tile([P, QT], F32, tag="l1")
                for k in range(4):
                    nc.tensor.matmul(ps[:], v1_s[:, k, bass.ts(m, P)], xq[:, k, :],
                                     start=(k == 0), stop=(k == 3))
                nc.scalar.activation(hv_t[:, m, :], ps[:], AF.Relu,
                                     bias=c1_s[:, m:m + 1])
            return {"q": q, "hid": hid_t, "hv": hv_t}

        def emit_l2(st):
            """L2: token-major blk; W2 streamed in quarter slabs."""
            q, hid_t = st["q"], st["hid"]
            blks = [blkpool.tile([P, NF], BF16, tag="blk", name=f"blk{q}_{i}")
                    for i in range(TPQ)]
            for n in range(NF // 512):
                pss = [l2ps.tile([P, 512], F32, tag="l2", name=f"l2ps{q}_{n}_{i}")
                       for i in range(TPQ)]
                for qtr in range(4):
                    w2q = w2pool.tile([P, 8, 512], BF16, tag="w2n",
                                      name=f"w2n{q}_{n}_{qtr}")
                    nc.sync.dma_start(
                        w2q[:], bass.AP(w2, (n * HID + 8 * qtr * P) * 512,
                                        [[512, P], [P * 512, 8], [1, 512]]))
                    for k8 in range(8):
                        k = qtr * 8 + k8
                        for ttq in range(TPQ):
                            nc.tensor.matmul(pss[ttq][:], hid_t[:, k, bass.ts(ttq, P)],
                                             w2q[:, k8, :], start=(k == 0),
                                             stop=(k == HID // P - 1))
                for ttq in range(TPQ):
                    nc.scalar.activation(blks[ttq][:, bass.ts(n, 512)], pss[ttq][:],
                                         AF.Identity)
            for ttq in range(TPQ):
                nc.vector.tensor_tensor(blks[ttq][:], blks[ttq][:], b2_s[:],
                                        ALU.add)
            st["blks"] = blks

        def emit_vnorm(st):
            """v2 psums + v write into at tiles (PE + small act)."""
            q, hv_t = st["q"], st["hv"]
            ats = [atpool.tile([P, NHO * AVW], BF16, tag="at", name=f"at{q}_{i}")
                   for i in range(TPQ)]
            for ttq in range(TPQ):
                psv = vps.tile([P, VF], F32, tag="v")
                nc.tensor.matmul(psv[:], ones_s[:1, :], c2_s[:1, :],
                                 start=True, stop=False)
                for k in range(4):
                    nc.tensor.matmul(psv[:], hv_t[:, k, bass.ts(ttq, P)],
                                     v2_s[:, k, :], start=False, stop=(k == 3))
                nc.scalar.activation(
                    bass.AP(ats[ttq].tensor, ats[ttq][:].offset + 8,
                            [[NHO * AVW, P], [72, HL], [9, BD]]),
                    bass.AP(psv.tensor, psv[:].offset, [[VF, P], [8, HL], [1, BD]]),
                    AF.Identity)
            st["ats"] = ats

        def emit_norm(st):
            """|blk|^1.2 norm + A write-out (DVE + Act, batched)."""
            q, blks, ats = st["q"], st["blks"], st["ats"]
            for hf in range(2):
                tts = (2 * hf, 2 * hf + 1)
                pws = [pwpool.tile([P, NF], BF16, tag="pw", name=f"pw{q}_{i}")
                       for i in tts]
                for i, ttq in enumerate(tts):
                    nc.vector.tensor_tensor(pws[i][:], blks[ttq][:], blks[ttq][:],
                                            ALU.mult)
                for pw in pws:
                    nc.scalar.activation(pw[:], pw[:], AF.Ln)
                for pw in pws:
                    nc.scalar.activation(pw[:], pw[:], AF.Exp, scale=0.6)
                rchs = []
                for i, ttq in enumerate(tts):
                    # sum over rows i -> pst[(h, k)]
                    pst = smpool.tile([P, HL * BD], F32, tag="pst")
                    with nc.allow_low_precision(reason="norm stats"):
                        nc.vector.tensor_reduce(
                            pst[:].rearrange("p (h k) -> p h k", h=HL, k=BD),
                            bass.AP(pws[i].tensor, pws[i][:].offset,
                                    [[NF, P], [64, HL], [1, BD], [8, BD]]),
                            axis=mybir.AxisListType.X, op=ALU.add)
                    # max_k commutes with ^(1/1.2); rc = dm^(-1/1.2)
                    dm = smpool.tile([P, HL], F32, tag="dm", name=f"dm{q}_{ttq}")
                    nc.vector.tensor_reduce(
                        dm[:].rearrange("p (h one) -> p h one", h=HL, one=1),
                        pst[:].rearrange("p (h k) -> p h k", h=HL, k=BD),
                        axis=mybir.AxisListType.X, op=ALU.max)
                    rchs.append(dm)
                for dm in rchs:
                    nc.scalar.activation(dm[:], dm[:], AF.Ln)
                rcbs = []
                for dm, ttq in zip(rchs, tts):
                    rch = smpool.tile([P, HL], BF16, tag="rch", name=f"rch{q}_{ttq}")
                    nc.scalar.activation(rch[:], dm[:], AF.Exp, scale=-1.0 / 1.2)
                    rcbs.append(rch)
                for rch, ttq in zip(rcbs, tts):
                    tau = q * TPQ + ttq
                    at = ats[ttq]
                    # A = blk * rc (broadcast over i, k) into 9-strided at slots
                    nc.vector.tensor_tensor(
                        bass.AP(at.tensor, at[:].offset,
                                [[NHO * AVW, P], [72, HL], [9, BD], [1, BD]]),
                        bass.AP(blks[ttq].tensor, blks[ttq][:].offset,
                                [[NF, P], [64, HL], [8, BD], [1, BD]]),
                        bass.AP(rch.tensor, rch[:].offset,
                                [[HL, P], [1, HL], [0, BD], [0, BD]]),
                        ALU.mult)
                    for ho in range(NHO):
                        eng = nc.sync if ho < 5 else nc.gpsimd
                        eng.dma_start(
                            bass.AP(av_dram, ho * ROWW + _rot(tau) * AVW,
                                    [[NHO * ROWW, K], [AVW, 8], [1, AVW]]),
                            bass.AP(at.tensor, at[:].offset + ho * AVW,
                                    [[NHO * AVW, P], [1, AVW]]))

        def emit_window(q):
            lo, hi = windows[q]
            for p in range(lo, hi):
                scan_step(p)
                if p + 1 in (32, 64, 96):
                    emit_out(p // 32)
            if q == NQ - 1:
                emit_out(3)

        # ======== software-pipelined emission: norm(q-1) under stage-A(q) ====
        prev = None
        for q in range(NQ):
            st = emit_part1(q)
            if prev is not None:
                emit_vnorm(prev)
            emit_l2(st)
            if prev is not None:
                emit_norm(prev)
                emit_window(prev["q"])
            prev = st
        emit_vnorm(prev)
        emit_norm(prev)
        emit_window(NQ - 1)

    nc.compile()
    return nc


# ---------------- host side ----------------

_NC_CACHE = {}


def _get_nc(TOK=SEQ):
    if TOK not in _NC_CACHE:
        _NC_CACHE[TOK] = build_nc(TOK=TOK)
    return _NC_CACHE[TOK]


def _stripe_tokens():
    """token index for MLP column (tau, c, j) order, flattened [NQ*QT]."""
    cols = np.zeros(SEQ, np.int64)
    i = 0
    for tau in range(SEQ // P):
        for c in range(K):
            for j in range(8):
                cols[i] = c * C + _rot(tau) + j
                i += 1
    return cols


def prep_shared(W1, b1, W2, b2, V1, c1, V2, c2, a0):
    bf = ml_dtypes.bfloat16
    W2r = W2.reshape(H, BD, BD, HID)
    W2c = (W2r - W2r.mean(axis=1, keepdims=True)).reshape(H * BD * BD, HID)
    b2r = b2.reshape(H, BD, BD)
    b2c = (b2r - b2r.mean(axis=1, keepdims=True)).reshape(-1)
    shared = {
        "w1": np.ascontiguousarray(W1.T).astype(bf),
        "b1": np.asarray(b1).reshape(HID, 1).astype(np.float32),
        "v1": np.ascontiguousarray(V1.T).astype(bf),
        "c1": np.asarray(c1).reshape(EMB, 1).astype(np.float32),
    }
    halves = []
    for half in range(2):
        rsl = slice(half * NF, (half + 1) * NF)
        vsl = slice(half * VF, (half + 1) * VF)
        hsl = slice(half * HL, (half + 1) * HL)
        a0h = np.asarray(a0)[0, hsl]                       # [32, 8]
        a0p = a0h.reshape(NHO, NHR, BD).reshape(NHO, HRI)  # [ho, (hr, i)]
        w2h = np.ascontiguousarray(W2c[rsl].T).astype(bf)  # [HID, NF]
        w2n = np.ascontiguousarray(
            w2h.reshape(HID, NF // 512, 512).transpose(1, 0, 2)).reshape(-1)
        halves.append({
            "w2": w2n,
            "b2": np.ascontiguousarray(
                np.broadcast_to(b2c[rsl].reshape(1, NF), (P, NF))).astype(bf),
            "v2": np.ascontiguousarray(V2[vsl].T).astype(bf),
            "c2": np.asarray(c2)[vsl].reshape(1, VF).astype(bf),
            "a0": a0p.astype(bf),
        })
    return shared, halves


def make_in_maps(x, W1, b1, W2, b2, V1, c1, V2, c2, a0):
    shared, halves = prep_shared(W1, b1, W2, b2, V1, c1, V2, c2, a0)
    bf = ml_dtypes.bfloat16
    cols = _stripe_tokens()
    in_maps = []
    for core in range(N_CORES):
        b, half = core // 2, core % 2
        m = dict(shared)
        m.update(halves[half])
        xT = np.asarray(x)[b].T.astype(bf)            # [EMB, SEQ]
        xst = xT[:, cols]                             # striped columns
        # xs[k, p, q, col]
        m["xs"] = np.ascontiguousarray(
            xst.reshape(4, P, NQ_G, QT_G)).reshape(-1)
        in_maps.append(m)
    return in_maps


NQ_G = SEQ // 512
QT_G = 512


def kernel(x, W1, b1, W2, b2, V1, c1, V2, c2, a0):
    from concourse import bass_utils
    nc = _get_nc(SEQ)
    in_maps = make_in_maps(x, W1, b1, W2, b2, V1, c1, V2, c2, a0)
    res = bass_utils.run_bass_kernel_spmd(nc, in_maps, core_ids=list(range(N_CORES)))
    out = np.zeros((BS, SEQ, EMB), np.float32)
    for core in range(N_CORES):
        b, half = core // 2, core % 2
        raw = res.results[core]["out"].astype(np.float32)   # [128, C*HRI]
        o = raw.reshape(K, NHO, C, NHR, BD).transpose(0, 2, 1, 3, 4)
        out[b, :, half * VF:(half + 1) * VF] = o.reshape(SEQ, VF)
    return out
